# revision 23
# baseline (speedup 1.0000x reference)
"""GATv2 (2-layer) edge-phase kernel for 8 TRN2 NeuronCores — v5.

Per-layer edge phase, per core (destination-partitioned):
  * Nodes ranked by in-degree, dealt round-robin to cores (rank % 8), then
    blocked into 98 windows of 128 consecutive ranks.  Window w keeps its
    128 dst nodes on the 128 SBUF partitions; edges of a node occupy free
    slots 0..K_w-1 (K_w = max in-degree in the window, shared across cores).
  * Host ships *gathered* per-edge rows [v | bias] where v = xl[src]+xr[dst]
    is PRE-ADDED on the host and bias = 0.6*(al[src]+ar[dst]) (the linear
    part of lrelu(v) = 0.6 v + 0.4|v| dotted with att; al/ar are per-node
    att-dots).  Pad slots get bias = -1e4, so exp() masks them for free.
  * Device per window:
      Act:    a = |v|            (Abs and Exp share an activation table)
      DVE:    q = a * att4, qs = reduce_c(q), logits = qs + bias
      Act:    exp(logits) -> cat den slots, exp expanded over c -> wexp
      GpSimd: cat msg slots = v * wexp
      PE:     K identity matmuls accumulate [den | SUM w*v] in PSUM
      DVE:    PSUM -> SBUF, group-batched DMA out
    4-stage software pipeline; every engine streams independently.
  * Host removes the xr contamination after aggregation:
    SUM w*(xl+xr) = msg + den*xr  =>  out = MSG/den - xr.

Host: dense linears, gathers + pre-adds, ELU, normalization, log_softmax.
"""
import sys, os
sys.path.insert(0, "/opt/trn_rl_repo")
import numpy as np
import ml_dtypes

TRACE = bool(int(os.environ.get("BASS_KERNEL_TRACE", "0")))
EXEC_NS = []
TRACE_PATHS = []

if TRACE and "antenv.axon_hooks" not in sys.modules:
    try:
        import types
        from trn_agent_boot.trn_boot import _ntff_profile_via_ctypes
        _m = types.ModuleType("antenv.axon_hooks")
        _hook = _ntff_profile_via_ctypes("/opt/axon/libaxon_pjrt.so")
        _m.get_axon_ntff_profile_hook = lambda: _hook
        sys.modules["antenv.axon_hooks"] = _m
    except Exception as _e:
        print(f"trace hook setup failed: {_e}", file=sys.stderr)
        TRACE = False

import concourse.bass as bass
import concourse.bacc as bacc
import concourse.mybir as mybir
import concourse.tile as tile
from concourse.bass_utils import run_bass_kernel_spmd

# ---------------- problem constants ----------------
N = 100000
F_IN = 256
HID, H1, H2, NCLS = 8, 8, 4, 40
D1 = H1 * HID            # 64
D2 = H2 * NCLS           # 160
NCORES = 8
NJ = N // NCORES         # 12500 valid rows per core
W = (NJ + 127) // 128    # 98 windows per core
NC_N = W * 128           # 12544 rows incl pad

GROUP_BYTES = 20480      # per-partition budget for one gathered group

BF16 = ml_dtypes.bfloat16
AP = bass.AP

_cache = {}


def _v(t, off, *dims):
    b = t[:]
    return AP(b.tensor, b.offset + off, [b.ap[0], *dims])


def _groups(Kw, budget_cols):
    groups = []
    w0, acc = 0, 0
    for w in range(W):
        k = int(Kw[w])
        if acc and acc + k > budget_cols:
            groups.append((w0, w))
            w0, acc = w, 0
        acc += k
    groups.append((w0, W))
    return groups


def _build_edge_program(H, C, Kw):
    CHr = H * C
    RW = CHr + H             # gathered row: [v | bias]
    OUTW = H + CHr           # psum row: [den | msg]
    offs = np.concatenate(([0], np.cumsum(Kw))).astype(np.int64)
    SK = int(offs[-1])
    groups = _groups(Kw, max(GROUP_BYTES // (RW * 2), int(Kw.max())))
    ngroups = len(groups)
    grp_of = np.zeros(W, np.int64)
    for gi, (w0, w1) in enumerate(groups):
        grp_of[w0:w1] = gi

    Kmax = int(Kw.max())
    nc = bacc.Bacc("TRN2")
    f32, bf16 = mybir.dt.float32, mybir.dt.bfloat16
    gxl = nc.declare_dram_parameter("gxl", [128, SK * RW], bf16, isOutput=False)
    atr = nc.declare_dram_parameter("atr", [128, Kmax * CHr], bf16, isOutput=False)
    idn = nc.declare_dram_parameter("idn", [128, 128], bf16, isOutput=False)
    out = nc.declare_dram_parameter("out", [128, W * OUTW], f32, isOutput=True)

    EXPF = mybir.ActivationFunctionType.Exp
    ABSF = mybir.ActivationFunctionType.Abs
    ADD, MUL = mybir.AluOpType.add, mybir.AluOpType.mult

    with tile.TileContext(nc) as tc:
        with (
            tc.tile_pool(name="const", bufs=1) as pc,
            tc.tile_pool(name="grp", bufs=3) as pg,
            tc.tile_pool(name="a", bufs=3) as pa,
            tc.tile_pool(name="sm", bufs=3) as psm,
            tc.tile_pool(name="cat", bufs=3) as pcat,
            tc.tile_pool(name="wx", bufs=3) as pwx,
            tc.tile_pool(name="ob", bufs=2) as pob,
            tc.tile_pool(name="psum", bufs=3, space="PSUM") as pp,
        ):
            att_sb = pc.tile([128, Kmax * CHr], bf16, tag="att")
            idn_sb = pc.tile([128, 128], bf16, tag="idn")
            nc.sync.dma_start(out=att_sb[:], in_=atr[:])
            nc.sync.dma_start(out=idn_sb[:], in_=idn[:])

            gX = {}
            obg = {}
            wins = {}

            def load_group(g):
                w0, w1 = groups[g]
                c0, c1 = int(offs[w0]), int(offs[w1])
                gx = pg.tile([128, (c1 - c0) * RW], bf16, tag="gx")
                nc.sync.dma_start(out=gx[:], in_=gxl[:, c0 * RW:c1 * RW])
                gX[g] = (gx, c0)

            def stage_a(w):
                g = int(grp_of[w])
                if w == groups[g][0] and g + 1 < ngroups:
                    load_group(g + 1)
                K = int(Kw[w])
                gx, c0 = gX[g]
                base = (int(offs[w]) - c0) * RW
                a = pa.tile([128, K * CHr], bf16, tag="a")
                nc.scalar.activation(
                    out=_v(a, 0, (CHr, K), (1, CHr)),
                    in_=_v(gx, base, (RW, K), (1, CHr)), func=ABSF)
                wins[w] = dict(a=a, g=g, base=base, K=K)

            def stage_b(w):
                d = wins[w]
                K, a, g, base = d["K"], d["a"], d["g"], d["base"]
                gx, c0 = gX[g]
                # q = a * att4 (in place; att tiled K times -> contiguous in1)
                nc.vector.tensor_tensor(
                    out=_v(a, 0, (1, K * CHr)),
                    in0=_v(a, 0, (1, K * CHr)),
                    in1=_v(att_sb, 0, (1, K * CHr)), op=MUL)
                qs = psm.tile([128, K * H], f32, tag="qs")
                nc.vector.tensor_reduce(
                    out=_v(qs, 0, (H, K), (1, H)),
                    in_=_v(a, 0, (CHr, K), (C, H), (1, C)),
                    axis=mybir.AxisListType.X, op=ADD)
                lg = psm.tile([128, K * H], f32, tag="lg")
                nc.vector.tensor_tensor(
                    out=_v(lg, 0, (H, K), (1, H)),
                    in0=_v(qs, 0, (H, K), (1, H)),
                    in1=_v(gx, base + CHr, (RW, K), (1, H)), op=ADD)
                wx = pwx.tile([128, K * CHr], bf16, tag="wx")
                nc.scalar.activation(
                    out=wx[:], in_=_v(lg, 0, (H, K), (1, H), (0, C)), func=EXPF)
                d["wx"] = wx

            def stage_c(w):
                d = wins[w]
                K, wx, g, base = d["K"], d["wx"], d["g"], d["base"]
                gx, c0 = gX[g]
                cat = pcat.tile([128, K * OUTW], bf16, tag="cat")
                # den slots: copy w from the expanded exp tile (DVE)
                nc.vector.tensor_copy(
                    out=_v(cat, 0, (OUTW, K), (1, H)),
                    in_=_v(wx, 0, (CHr, K), (C, H)))
                # cat msg slots = v * wexp  (GpSimd)
                nc.gpsimd.tensor_tensor(
                    out=_v(cat, H, (OUTW, K), (C, H), (1, C)),
                    in0=_v(gx, base, (RW, K), (C, H), (1, C)),
                    in1=_v(wx, 0, (CHr, K), (C, H), (1, C)), op=MUL)
                ps = pp.tile([128, OUTW], f32, tag="ps")
                cb = cat[:]
                for k in range(K):
                    nc.tensor.matmul(
                        out=ps[:], lhsT=idn_sb[:],
                        rhs=AP(cb.tensor, cb.offset + k * OUTW,
                               [cb.ap[0], (1, OUTW)]),
                        start=(k == 0), stop=(k == K - 1))
                d["ps"] = ps

            def stage_d(w):
                d = wins.pop(w)
                ps = d["ps"]
                g2 = int(grp_of[w])
                w0, w1 = groups[g2]
                if w == w0:
                    obg[g2] = pob.tile([128, (w1 - w0) * OUTW], f32,
                                       name="ob", tag="ob")
                nc.vector.tensor_copy(
                    out=_v(obg[g2], (w - w0) * OUTW, (1, OUTW)), in_=ps[:])
                if w == w1 - 1:
                    nc.sync.dma_start(
                        out=out[:, w0 * OUTW:w1 * OUTW], in_=obg[g2][:])

            load_group(0)
            stage_a(0)
            for w in range(W):
                if w + 1 < W:
                    stage_a(w + 1)
                stage_b(w)
                if w >= 1:
                    stage_c(w - 1)
                if w >= 2:
                    stage_d(w - 2)
            stage_c(W - 1)
            stage_d(W - 2)
            stage_d(W - 1)
    nc.compile()
    return nc, SK


def _prep_graph(src, dst):
    """Degree-ranked window assignment + per-(core,window) edge slotting."""
    deg = np.bincount(dst, minlength=N)          # includes self-loops
    order = np.argsort(-deg, kind="stable").astype(np.int64)
    rank = np.empty(N, np.int64)
    rank[order] = np.arange(N)
    core_of = rank % NCORES
    j = rank // NCORES
    w_of = j // 128
    pos_of = j % 128

    Kcw = np.zeros((NCORES, W), np.int64)
    np.maximum.at(Kcw, (core_of, w_of), deg)
    Kw = Kcw.max(axis=0)                          # [W] shared across cores
    offs = np.concatenate(([0], np.cumsum(Kw)))
    SK = int(offs[-1])

    ne = dst.size
    sidx = np.argsort(dst, kind="stable")
    sd = dst[sidx]
    cum = np.concatenate(([0], np.cumsum(deg)))
    k_sorted = np.arange(ne) - cum[sd]
    k_e = np.empty(ne, np.int64)
    k_e[sidx] = k_sorted

    c_e = core_of[dst]
    p_e = pos_of[dst]
    col_e = offs[w_of[dst]] + k_e

    idx_flat = np.full((NCORES, 128, SK), N, np.int32)   # N = sentinel row
    idx_flat[c_e, p_e, col_e] = src.astype(np.int32)

    wcol = np.repeat(np.arange(W), Kw)            # window id of each column
    jj = np.arange(NJ)
    node_of = order[jj[None, :] * NCORES + np.arange(NCORES)[:, None]]
    return dict(Kw=Kw, idx_flat=idx_flat, node_of=node_of, wcol=wcol)


def _run_layer(gp, xl, xr, att, H, C):
    CHr = H * C
    RW = CHr + H
    OUTW = H + CHr
    SK = gp["idx_flat"].shape[-1]
    attm = att.reshape(H, C)
    al = 0.6 * np.einsum('nhc,hc->nh', xl.reshape(N, H, C), attm)
    ar = 0.6 * np.einsum('nhc,hc->nh', xr.reshape(N, H, C), attm)
    tab = np.zeros((N + 1, RW), np.float32)
    tab[:N, :CHr] = xl
    tab[:N, CHr:] = al
    tab[N, CHr:] = -1e4
    Kmax = int(gp["Kw"].max())
    att_r = np.tile((0.4 * att).reshape(1, CHr).astype(BF16), (128, Kmax))
    iden = np.eye(128, dtype=np.float32).astype(BF16)

    in_maps = []
    for c in range(NCORES):
        nodes = gp["node_of"][c]
        # per-window dst-side row to pre-add: [xr | ar]
        xrb = np.zeros((NC_N, RW), np.float32)
        xrb[:NJ, :CHr] = xr[nodes]
        xrb[:NJ, CHr:] = ar[nodes]
        xrb = xrb.reshape(W, 128, RW).transpose(1, 0, 2)   # [128, W, RW]
        g3 = tab[gp["idx_flat"][c]]                        # [128, SK, RW] f32
        g3 += xrb[:, gp["wcol"], :]
        in_maps.append(dict(
            gxl=g3.astype(BF16).reshape(128, SK * RW), atr=att_r, idn=iden))

    key = (H, C, tuple(gp["Kw"].tolist()))
    if key not in _cache:
        _cache[key] = _build_edge_program(H, C, gp["Kw"])
    nc, _ = _cache[key]
    res = run_bass_kernel_spmd(nc, in_maps, list(range(NCORES)), trace=TRACE)
    if TRACE:
        EXEC_NS.append(res.exec_time_ns)
        if res.instructions_and_trace:
            TRACE_PATHS.append(res.instructions_and_trace[1])

    den = np.zeros((N, H), np.float32)
    msg = np.zeros((N, CHr), np.float32)
    for c in range(NCORES):
        o = res.results[c]["out"].reshape(128, W, OUTW).transpose(1, 0, 2)
        o = o.reshape(NC_N, OUTW)[:NJ]
        nodes = gp["node_of"][c]
        den[nodes] = o[:, :H]
        msg[nodes] = o[:, H:]
    return den, msg


def kernel(x, edge_index, Wl1, bl1, Wr1, br1, att1, b1,
           Wl2, bl2, Wr2, br2, att2, b2):
    x = np.asarray(x, np.float32)
    ei = np.asarray(edge_index).astype(np.int64)
    loop = np.arange(N, dtype=np.int64)
    src = np.concatenate([ei[0], loop])
    dst = np.concatenate([ei[1], loop])
    gp = _prep_graph(src, dst)

    xl1 = x @ np.asarray(Wl1, np.float32) + np.asarray(bl1, np.float32)
    xr1 = x @ np.asarray(Wr1, np.float32) + np.asarray(br1, np.float32)
    den1, msg1 = _run_layer(gp, xl1, xr1, np.asarray(att1, np.float32), H1, HID)
    # device summed w*(xl+xr): subtract den*xr
    out1 = msg1.reshape(N, H1, HID) / (den1[:, :, None] + 1e-16) \
        - xr1.reshape(N, H1, HID)
    h = out1.reshape(N, D1) + np.asarray(b1, np.float32)
    h = np.where(h > 0, h, np.expm1(h))          # ELU

    xl2 = h @ np.asarray(Wl2, np.float32) + np.asarray(bl2, np.float32)
    xr2 = h @ np.asarray(Wr2, np.float32) + np.asarray(br2, np.float32)
    den2, msg2 = _run_layer(gp, xl2, xr2, np.asarray(att2, np.float32), H2, NCLS)
    out2 = msg2.reshape(N, H2, NCLS) / (den2[:, :, None] + 1e-16) \
        - xr2.reshape(N, H2, NCLS)
    o = out2.mean(1) + np.asarray(b2, np.float32)
    o = o - o.max(1, keepdims=True)
    o = o - np.log(np.exp(o).sum(1, keepdims=True))
    return o.astype(np.float32)


# revision 26
# speedup vs baseline: 1.2433x; 1.2433x over previous
"""GATv2 (2-layer) edge-phase kernel for 8 TRN2 NeuronCores — v5.

Per-layer edge phase, per core (destination-partitioned):
  * Nodes ranked by in-degree, dealt round-robin to cores (rank % 8), then
    blocked into 98 windows of 128 consecutive ranks.  Window w keeps its
    128 dst nodes on the 128 SBUF partitions; edges of a node occupy free
    slots 0..K_w-1 (K_w = max in-degree in the window, shared across cores).
  * Host ships *gathered* per-edge rows [v | bias] where v = xl[src]+xr[dst]
    is PRE-ADDED on the host and bias = 0.6*(al[src]+ar[dst]) (the linear
    part of lrelu(v) = 0.6 v + 0.4|v| dotted with att; al/ar are per-node
    att-dots).  Pad slots get bias = -1e4, so exp() masks them for free.
  * Device per window:
      Act:    a = |v|            (Abs and Exp share an activation table)
      DVE:    q = a * att4, qs = reduce_c(q), logits = qs + bias
      Act:    exp(logits) -> cat den slots, exp expanded over c -> wexp
      GpSimd: cat msg slots = v * wexp
      PE:     K identity matmuls accumulate [den | SUM w*v] in PSUM
      DVE:    PSUM -> SBUF, group-batched DMA out
    4-stage software pipeline; every engine streams independently.
  * Host removes the xr contamination after aggregation:
    SUM w*(xl+xr) = msg + den*xr  =>  out = MSG/den - xr.

Host: dense linears, gathers + pre-adds, ELU, normalization, log_softmax.
"""
import sys, os
sys.path.insert(0, "/opt/trn_rl_repo")
import numpy as np
import ml_dtypes

TRACE = bool(int(os.environ.get("BASS_KERNEL_TRACE", "0")))
EXEC_NS = []
TRACE_PATHS = []

if "antenv.axon_hooks" not in sys.modules:
    # this image's antenv lacks axon_hooks; register the ctypes NTFF hook
    # so run_bass_kernel_spmd(trace=True) / BASS_TRACE=1 works under axon.
    try:
        import types
        import antenv  # noqa: F401  (parent package must import first)
        from trn_agent_boot.trn_boot import _ntff_profile_via_ctypes
        _m = types.ModuleType("antenv.axon_hooks")
        _hook = _ntff_profile_via_ctypes("/opt/axon/libaxon_pjrt.so")
        _m.get_axon_ntff_profile_hook = lambda: _hook
        sys.modules["antenv.axon_hooks"] = _m
    except Exception as _e:
        if TRACE:
            print(f"trace hook setup failed: {_e}", file=sys.stderr)
            TRACE = False

import concourse.bass as bass
import concourse.bacc as bacc
import concourse.mybir as mybir
import concourse.tile as tile
from concourse.bass_utils import run_bass_kernel_spmd

# ---------------- problem constants ----------------
N = 100000
F_IN = 256
HID, H1, H2, NCLS = 8, 8, 4, 40
D1 = H1 * HID            # 64
D2 = H2 * NCLS           # 160
NCORES = 8
NJ = N // NCORES         # 12500 valid rows per core
W = (NJ + 127) // 128    # 98 windows per core
NC_N = W * 128           # 12544 rows incl pad

GROUP_BYTES = 20480      # per-partition budget for one gathered group

BF16 = ml_dtypes.bfloat16
AP = bass.AP

_cache = {}


def _v(t, off, *dims):
    b = t[:]
    return AP(b.tensor, b.offset + off, [b.ap[0], *dims])


def _groups(Kw, budget_cols):
    groups = []
    w0, acc = 0, 0
    for w in range(W):
        k = int(Kw[w])
        if acc and acc + k > budget_cols:
            groups.append((w0, w))
            w0, acc = w, 0
        acc += k
    groups.append((w0, W))
    return groups


def _build_edge_program(H, C, Kw):
    CHr = H * C
    RW = CHr + H             # gathered row: [v | bias]
    OUTW = H + CHr           # psum row: [den | msg]
    offs = np.concatenate(([0], np.cumsum(Kw))).astype(np.int64)
    SK = int(offs[-1])
    groups = _groups(Kw, max(GROUP_BYTES // (RW * 2), int(Kw.max())))
    ngroups = len(groups)
    grp_of = np.zeros(W, np.int64)
    for gi, (w0, w1) in enumerate(groups):
        grp_of[w0:w1] = gi

    Kmax = int(Kw.max())
    nc = bacc.Bacc("TRN2")
    f32, bf16 = mybir.dt.float32, mybir.dt.bfloat16
    gxl = nc.declare_dram_parameter("gxl", [128, SK * RW], bf16, isOutput=False)
    atr = nc.declare_dram_parameter("atr", [128, Kmax * CHr], bf16, isOutput=False)
    idn = nc.declare_dram_parameter("idn", [128, 128], bf16, isOutput=False)
    out = nc.declare_dram_parameter("out", [128, W * OUTW], f32, isOutput=True)

    EXPF = mybir.ActivationFunctionType.Exp
    ABSF = mybir.ActivationFunctionType.Abs
    ADD, MUL = mybir.AluOpType.add, mybir.AluOpType.mult

    with tile.TileContext(nc) as tc:
        with (
            tc.tile_pool(name="const", bufs=1) as pc,
            tc.tile_pool(name="grp", bufs=3) as pg,
            tc.tile_pool(name="a", bufs=3) as pa,
            tc.tile_pool(name="sm", bufs=3) as psm,
            tc.tile_pool(name="cat", bufs=3) as pcat,
            tc.tile_pool(name="wx", bufs=3) as pwx,
            tc.tile_pool(name="ob", bufs=2) as pob,
            tc.tile_pool(name="psum", bufs=3, space="PSUM") as pp,
        ):
            att_sb = pc.tile([128, Kmax * CHr], bf16, tag="att")
            idn_sb = pc.tile([128, 128], bf16, tag="idn")
            nc.sync.dma_start(out=att_sb[:], in_=atr[:])
            nc.sync.dma_start(out=idn_sb[:], in_=idn[:])

            gX = {}
            obg = {}
            wins = {}

            def load_group(g):
                w0, w1 = groups[g]
                c0, c1 = int(offs[w0]), int(offs[w1])
                gx = pg.tile([128, (c1 - c0) * RW], bf16, tag="gx")
                nc.sync.dma_start(out=gx[:], in_=gxl[:, c0 * RW:c1 * RW])
                gX[g] = (gx, c0)

            def stage_a(w):
                g = int(grp_of[w])
                if w == groups[g][0] and g + 1 < ngroups:
                    load_group(g + 1)
                K = int(Kw[w])
                gx, c0 = gX[g]
                base = (int(offs[w]) - c0) * RW
                a = pa.tile([128, K * CHr], bf16, tag="a")
                nc.scalar.activation(
                    out=_v(a, 0, (CHr, K), (1, CHr)),
                    in_=_v(gx, base, (RW, K), (1, CHr)), func=ABSF)
                wins[w] = dict(a=a, g=g, base=base, K=K)

            def stage_b(w):
                d = wins[w]
                K, a, g, base = d["K"], d["a"], d["g"], d["base"]
                gx, c0 = gX[g]
                # q = a * att4 (in place; att tiled K times -> contiguous in1)
                nc.vector.tensor_tensor(
                    out=_v(a, 0, (1, K * CHr)),
                    in0=_v(a, 0, (1, K * CHr)),
                    in1=_v(att_sb, 0, (1, K * CHr)), op=MUL)
                qs = psm.tile([128, K * H], f32, tag="qs")
                nc.vector.tensor_reduce(
                    out=_v(qs, 0, (H, K), (1, H)),
                    in_=_v(a, 0, (CHr, K), (C, H), (1, C)),
                    axis=mybir.AxisListType.X, op=ADD)
                lg = psm.tile([128, K * H], f32, tag="lg")
                nc.vector.tensor_tensor(
                    out=_v(lg, 0, (H, K), (1, H)),
                    in0=_v(qs, 0, (H, K), (1, H)),
                    in1=_v(gx, base + CHr, (RW, K), (1, H)), op=ADD)
                cat = pcat.tile([128, K * OUTW], bf16, tag="cat")
                nc.scalar.activation(
                    out=_v(cat, 0, (OUTW, K), (1, H)), in_=lg[:], func=EXPF)
                wx = pwx.tile([128, K * CHr], bf16, tag="wx")
                nc.scalar.activation(
                    out=wx[:], in_=_v(lg, 0, (H, K), (1, H), (0, C)), func=EXPF)
                d["cat"] = cat
                d["wx"] = wx

            def stage_c(w):
                d = wins[w]
                K, cat, wx, g, base = d["K"], d["cat"], d["wx"], d["g"], d["base"]
                gx, c0 = gX[g]
                # cat msg slots = v * wexp  (~90% GpSimd, rest DVE for balance)
                eng = nc.vector if w % 10 == 0 else nc.gpsimd
                eng.tensor_tensor(
                    out=_v(cat, H, (OUTW, K), (C, H), (1, C)),
                    in0=_v(gx, base, (RW, K), (C, H), (1, C)),
                    in1=_v(wx, 0, (CHr, K), (C, H), (1, C)), op=MUL)
                ps = pp.tile([128, OUTW], f32, tag="ps")
                cb = cat[:]
                for k in range(K):
                    nc.tensor.matmul(
                        out=ps[:], lhsT=idn_sb[:],
                        rhs=AP(cb.tensor, cb.offset + k * OUTW,
                               [cb.ap[0], (1, OUTW)]),
                        start=(k == 0), stop=(k == K - 1))
                d["ps"] = ps

            def stage_d(w):
                d = wins.pop(w)
                ps = d["ps"]
                g2 = int(grp_of[w])
                w0, w1 = groups[g2]
                if w == w0:
                    obg[g2] = pob.tile([128, (w1 - w0) * OUTW], f32,
                                       name="ob", tag="ob")
                nc.vector.tensor_copy(
                    out=_v(obg[g2], (w - w0) * OUTW, (1, OUTW)), in_=ps[:])
                if w == w1 - 1:
                    nc.sync.dma_start(
                        out=out[:, w0 * OUTW:w1 * OUTW], in_=obg[g2][:])

            load_group(0)
            stage_a(0)
            for w in range(W):
                if w + 1 < W:
                    stage_a(w + 1)
                stage_b(w)
                if w >= 1:
                    stage_c(w - 1)
                if w >= 2:
                    stage_d(w - 2)
            stage_c(W - 1)
            stage_d(W - 2)
            stage_d(W - 1)
    nc.compile()
    return nc, SK


def _prep_graph(src, dst):
    """Degree-ranked window assignment + per-(core,window) edge slotting."""
    deg = np.bincount(dst, minlength=N)          # includes self-loops
    order = np.argsort(-deg, kind="stable").astype(np.int64)
    rank = np.empty(N, np.int64)
    rank[order] = np.arange(N)
    core_of = rank % NCORES
    j = rank // NCORES
    w_of = j // 128
    pos_of = j % 128

    Kcw = np.zeros((NCORES, W), np.int64)
    np.maximum.at(Kcw, (core_of, w_of), deg)
    Kw = Kcw.max(axis=0)                          # [W] shared across cores
    offs = np.concatenate(([0], np.cumsum(Kw)))
    SK = int(offs[-1])

    ne = dst.size
    sidx = np.argsort(dst, kind="stable")
    sd = dst[sidx]
    cum = np.concatenate(([0], np.cumsum(deg)))
    k_sorted = np.arange(ne) - cum[sd]
    k_e = np.empty(ne, np.int64)
    k_e[sidx] = k_sorted

    c_e = core_of[dst]
    p_e = pos_of[dst]
    col_e = offs[w_of[dst]] + k_e

    idx_flat = np.full((NCORES, 128, SK), N, np.int32)   # N = sentinel row
    idx_flat[c_e, p_e, col_e] = src.astype(np.int32)

    wcol = np.repeat(np.arange(W), Kw)            # window id of each column
    jj = np.arange(NJ)
    node_of = order[jj[None, :] * NCORES + np.arange(NCORES)[:, None]]
    return dict(Kw=Kw, idx_flat=idx_flat, node_of=node_of, wcol=wcol)


def _run_layer(gp, xl, xr, att, H, C):
    CHr = H * C
    RW = CHr + H
    OUTW = H + CHr
    SK = gp["idx_flat"].shape[-1]
    attm = att.reshape(H, C)
    al = 0.6 * np.einsum('nhc,hc->nh', xl.reshape(N, H, C), attm)
    ar = 0.6 * np.einsum('nhc,hc->nh', xr.reshape(N, H, C), attm)
    tab = np.zeros((N + 1, RW), np.float32)
    tab[:N, :CHr] = xl
    tab[:N, CHr:] = al
    tab[N, CHr:] = -1e4
    Kmax = int(gp["Kw"].max())
    att_r = np.tile((0.4 * att).reshape(1, CHr).astype(BF16), (128, Kmax))
    iden = np.eye(128, dtype=np.float32).astype(BF16)

    in_maps = []
    for c in range(NCORES):
        nodes = gp["node_of"][c]
        # per-window dst-side row to pre-add: [xr | ar]
        xrb = np.zeros((NC_N, RW), np.float32)
        xrb[:NJ, :CHr] = xr[nodes]
        xrb[:NJ, CHr:] = ar[nodes]
        xrb = xrb.reshape(W, 128, RW).transpose(1, 0, 2)   # [128, W, RW]
        g3 = tab[gp["idx_flat"][c]]                        # [128, SK, RW] f32
        g3 += xrb[:, gp["wcol"], :]
        in_maps.append(dict(
            gxl=g3.astype(BF16).reshape(128, SK * RW), atr=att_r, idn=iden))

    key = (H, C, tuple(gp["Kw"].tolist()))
    if key not in _cache:
        _cache[key] = _build_edge_program(H, C, gp["Kw"])
    nc, _ = _cache[key]
    res = run_bass_kernel_spmd(nc, in_maps, list(range(NCORES)), trace=TRACE)
    if TRACE:
        EXEC_NS.append(res.exec_time_ns)
        if res.instructions_and_trace:
            TRACE_PATHS.append(res.instructions_and_trace[1])

    den = np.zeros((N, H), np.float32)
    msg = np.zeros((N, CHr), np.float32)
    for c in range(NCORES):
        o = res.results[c]["out"].reshape(128, W, OUTW).transpose(1, 0, 2)
        o = o.reshape(NC_N, OUTW)[:NJ]
        nodes = gp["node_of"][c]
        den[nodes] = o[:, :H]
        msg[nodes] = o[:, H:]
    return den, msg


def kernel(x, edge_index, Wl1, bl1, Wr1, br1, att1, b1,
           Wl2, bl2, Wr2, br2, att2, b2):
    x = np.asarray(x, np.float32)
    ei = np.asarray(edge_index).astype(np.int64)
    loop = np.arange(N, dtype=np.int64)
    src = np.concatenate([ei[0], loop])
    dst = np.concatenate([ei[1], loop])
    gp = _prep_graph(src, dst)

    xl1 = x @ np.asarray(Wl1, np.float32) + np.asarray(bl1, np.float32)
    xr1 = x @ np.asarray(Wr1, np.float32) + np.asarray(br1, np.float32)
    den1, msg1 = _run_layer(gp, xl1, xr1, np.asarray(att1, np.float32), H1, HID)
    # device summed w*(xl+xr): subtract den*xr
    out1 = msg1.reshape(N, H1, HID) / (den1[:, :, None] + 1e-16) \
        - xr1.reshape(N, H1, HID)
    h = out1.reshape(N, D1) + np.asarray(b1, np.float32)
    h = np.where(h > 0, h, np.expm1(h))          # ELU

    xl2 = h @ np.asarray(Wl2, np.float32) + np.asarray(bl2, np.float32)
    xr2 = h @ np.asarray(Wr2, np.float32) + np.asarray(br2, np.float32)
    den2, msg2 = _run_layer(gp, xl2, xr2, np.asarray(att2, np.float32), H2, NCLS)
    out2 = msg2.reshape(N, H2, NCLS) / (den2[:, :, None] + 1e-16) \
        - xr2.reshape(N, H2, NCLS)
    o = out2.mean(1) + np.asarray(b2, np.float32)
    o = o - o.max(1, keepdims=True)
    o = o - np.log(np.exp(o).sum(1, keepdims=True))
    return o.astype(np.float32)
